# revision 1
# baseline (speedup 1.0000x reference)
"""Deformable feature-enhancement module (two deformable-conv branches +
plain 3x3 conv + 1x1 fusion + residual) on 8 Trainium2 NeuronCores.

Sharding: data-parallel over (batch=2) x (4 h-slabs of 64 rows) = 8 shards.
Each core computes out[b, :, h0:h0+64, :] from a zero-padded input slab.

Device algorithm (per core), all contractions on the TensorEngine:
  1. one GEMM-conv (9 shifted matmuls, PSUM-accum) -> [cm(64) ; offsets(36)]
  2. bilinear weights via the "hat" identity (offsets verified |off|<1):
     weight attached to window row (h+pnx+j), j in {0,1,2}:
       j=0: max(0,-off), j=1: 1-|off|, j=2: max(0,off)
     plus exact border fixups reproducing the reference's index clipping.
  3. x_off accumulated over the 9 (jx,jy) static shifts of the padded
     input, scaled per-position (scalar_tensor_tensor, weight fields
     transposed to per-partition scalars).
  4. wc contraction, concat, 1x1 fusion conv, bias, residual.
"""

import os
import numpy as np
import ml_dtypes

BF16 = ml_dtypes.bfloat16

# ---- problem constants (hardcoded; kernel.py must be self-contained) ----
B, C, H, W = 2, 64, 256, 256
N_CORES = 8
HS = H // 4            # 64 output rows per slab
POS = HS * W           # 16384 positions per core
RS, QS = 68, 260       # slab buffer rows/cols (image padded by 2)
P2 = RS * QS           # 17680
BLK = 1024             # positions per block (4 output rows)
NBLK = POS // BLK      # 16
TPB = BLK // 128       # 8 tiles of 128 positions per block
HGRP = 2048            # hat-group positions
NHG = POS // HGRP      # 8
FT = 512               # conv free-tile (2 output rows)

_CACHE = {}


def _emit_device_program(tc, nc, bass, mybir, D):
    from contextlib import ExitStack

    f32 = mybir.dt.float32
    bf16 = mybir.dt.bfloat16
    Alu = mybir.AluOpType
    Act = mybir.ActivationFunctionType

    ctx = ExitStack()
    with ctx:
        const = ctx.enter_context(tc.tile_pool(name="const", bufs=1))
        xs_pool = ctx.enter_context(tc.tile_pool(name="xsp", bufs=1))
        ps_conv = ctx.enter_context(
            tc.tile_pool(name="psconv", bufs=2, space="PSUM"))
        cm_pool = ctx.enter_context(tc.tile_pool(name="cmsb", bufs=3))
        offc_pool = ctx.enter_context(tc.tile_pool(name="offc", bufs=2))
        w_pool = ctx.enter_context(tc.tile_pool(name="wfld", bufs=2))
        tmp_pool = ctx.enter_context(tc.tile_pool(name="wtmp", bufs=2))
        fix_pool = ctx.enter_context(tc.tile_pool(name="fix", bufs=4))
        ps_tr = ctx.enter_context(
            tc.tile_pool(name="pstr", bufs=2, space="PSUM"))
        wallt_pool = ctx.enter_context(tc.tile_pool(name="wallt", bufs=3))
        wpt_pool = ctx.enter_context(tc.tile_pool(name="wptp", bufs=3))
        x25_pool = ctx.enter_context(tc.tile_pool(name="x25p", bufs=2))
        xoff_pool = ctx.enter_context(tc.tile_pool(name="xoffp", bufs=3))
        xofft_pool = ctx.enter_context(tc.tile_pool(name="xofftp", bufs=12))
        ps_g = ctx.enter_context(
            tc.tile_pool(name="psg", bufs=2, space="PSUM"))
        cat_pool = ctx.enter_context(tc.tile_pool(name="catp", bufs=2))
        ps_o = ctx.enter_context(
            tc.tile_pool(name="pso", bufs=2, space="PSUM"))
        io_pool = ctx.enter_context(tc.tile_pool(name="iop", bufs=2))

        # ---------------- load constants ----------------
        wconv_sb = const.tile([C, 9, 128], bf16)
        nc.sync.dma_start(wconv_sb[:], D["wconv"].ap())
        bconv_sb = const.tile([128, 1], f32)
        nc.sync.dma_start(bconv_sb[:], D["bconv"].ap())
        flag_sb = const.tile([128, 4], f32)
        nc.sync.dma_start(flag_sb[:], D["flag"].ap())
        wcx_sb = const.tile([128, 5, 128], bf16)
        nc.sync.dma_start(
            wcx_sb[:], bass.AP(D["wcx"], 0,
                               [[128, 128], [128 * 128, 5], [1, 128]]))
        wcy_sb = const.tile([128, 5, 64], bf16)
        nc.sync.dma_start(
            wcy_sb[:], bass.AP(D["wcy"], 0,
                               [[64, 128], [128 * 64, 5], [1, 64]]))
        wo1_sb = const.tile([128, 64], bf16)
        nc.sync.dma_start(wo1_sb[:], D["wo1"].ap())
        wo2_sb = const.tile([64, 64], bf16)
        nc.sync.dma_start(wo2_sb[:], D["wo2"].ap())
        bo_sb = const.tile([64, 1], f32)
        nc.sync.dma_start(bo_sb[:], D["bo"].ap())
        ident_sb = const.tile([128, 128], bf16)
        nc.sync.dma_start(ident_sb[:], D["ident"].ap())

        xs_sb = xs_pool.tile([C, RS, QS], bf16)
        nc.sync.dma_start(xs_sb[:], D["xs"].ap())

        cm_ap = D["cm_scratch"].ap()
        xsT_d = D["xsT"]

        # off-channel partition map (band 64:100):
        # 64:73 bx_offx (tap-natural), 73:82 bx_offy (pny-major),
        # 82:91 by_offx, 91:100 by_offy (pny-major)

        # ---------------- phase 1: fused conv ----------------
        offc_tiles = []
        for g in range(NHG):
            offc_g = offc_pool.tile([128, HGRP], bf16, tag="offc")
            offc_tiles.append(offc_g)
            for f in range(4):
                ft = g * 4 + f
                ps = ps_conv.tile([128, FT], f32, tag="psconv")
                r0 = 2 * ft + 1
                for k in range(9):
                    kx, ky = k // 3, k % 3
                    rhs = xs_sb[:, r0 + kx:r0 + kx + 2, 1 + ky:257 + ky]
                    nc.tensor.matmul(ps[:], wconv_sb[:, k, :], rhs,
                                     start=(k == 0), stop=(k == 8))
                cm_t = cm_pool.tile([64, FT], bf16, tag="cmsb")
                nc.scalar.activation(cm_t[:], ps[0:64, :], Act.Identity,
                                     bias=bconv_sb[0:64, :])
                nc.sync.dma_start(cm_ap[:, ft * FT:(ft + 1) * FT], cm_t[:])
                nc.scalar.activation(offc_g[64:100, f * FT:(f + 1) * FT],
                                     ps[64:100, :], Act.Identity,
                                     bias=bconv_sb[64:100, :])

        # ---------------- phase 2: hat weights + fixups ----------------
        def band(tile_, p0, p1, free_off, free_ap):
            a = tile_[p0:p1, :]
            return bass.AP(a.tensor, a.offset + free_off,
                           [[a.ap[0][0], p1 - p0]] + [list(x) for x in free_ap])

        wj_tiles = []
        for g in range(NHG):
            offc_g = offc_tiles[g]
            o = offc_g[64:100, :]
            # clamp offsets to [-1, 1] (rare overflows host-corrected);
            # in-place, sign preserved so fixup conditions stay exact
            nc.vector.tensor_scalar(o, o, -1.0, 1.0, Alu.max, Alu.min)
            noff = tmp_pool.tile([128, HGRP], bf16, tag="noff")
            nc.vector.tensor_scalar(noff[64:100, :], o, -1.0, None, Alu.mult)
            absv = tmp_pool.tile([128, HGRP], bf16, tag="absv")
            nc.vector.tensor_tensor(absv[64:100, :], o, noff[64:100, :],
                                    Alu.max)
            w0 = w_pool.tile([128, HGRP], bf16, tag="w0")
            w1 = w_pool.tile([128, HGRP], bf16, tag="w1")
            w2 = w_pool.tile([128, HGRP], bf16, tag="w2")
            nc.vector.tensor_scalar(w1[64:100, :], absv[64:100, :], -1.0, 1.0,
                                    Alu.mult, Alu.add)
            nc.vector.tensor_scalar(w0[64:100, :], noff[64:100, :], 0.0, None,
                                    Alu.max)
            nc.vector.tensor_scalar(w2[64:100, :], o, 0.0, None, Alu.max)
            wj_tiles.append((w0, w1, w2))

            nrg = HGRP // W   # 8 output rows per group
            # fixup types: 0=x-lo (h=0), 1=x-hi (h=255), 2=y-lo (w=0),
            # 3=y-hi (w=255); row selection via flag_sb column per type
            fixes = []
            if g == 0:
                fixes.append((0, 0, [[1, 256]], Alu.is_lt))
            if g == NHG - 1:
                fixes.append((1, HGRP - 256, [[1, 256]], Alu.is_ge))
            fixes += [(2, 0, [[W, nrg]], Alu.is_lt),
                      (3, W - 1, [[W, nrg]], Alu.is_ge)]
            for ftype, fo, fap, op in fixes:
                nel = int(np.prod([c for _, c in fap]))
                ms = fix_pool.tile([128, 256], bf16, tag="fixm")
                dd = fix_pool.tile([128, 256], bf16, tag="fixd")
                o_sl = band(offc_g, 64, 100, fo, fap)
                w1_sl = band(w1, 64, 100, fo, fap)
                m_sl = band(ms, 64, 100, 0, [[1, nel]])
                d_sl = band(dd, 64, 100, 0, [[1, nel]])
                nc.vector.tensor_scalar(m_sl, o_sl, 0.0, None, op)
                nc.vector.tensor_scalar(d_sl, w1_sl, -1.0, 2.0,
                                        Alu.mult, Alu.add)
                nc.vector.tensor_tensor(d_sl, d_sl, m_sl, Alu.mult)
                nc.vector.scalar_tensor_tensor(
                    w1_sl, d_sl, flag_sb[64:100, ftype:ftype + 1],
                    w1_sl, Alu.mult, Alu.add)

        # -------- phase 3: transpose weight fields + products ----------
        wpt_all = []
        for kb in range(NBLK):
            wpt = wpt_pool.tile([128, TPB, 162], f32, tag="wpt")
            wpt_all.append(wpt)
            for t in range(TPB):
                tt = kb * TPB + t
                g, tg = tt // 16, tt % 16
                w0, w1, w2 = wj_tiles[g]
                pst = ps_tr.tile([128, 112], bf16, tag="pstr")
                for j, wt in enumerate((w0, w1, w2)):
                    nc.tensor.transpose(
                        pst[:, j * 36:(j + 1) * 36],
                        wt[64:100, tg * 128:(tg + 1) * 128],
                        ident_sb[64:100, 0:36])
                wat = wallt_pool.tile([128, 108], bf16, tag="wallt")
                nc.scalar.activation(wat[:], pst[:, 0:108], Act.Copy)
                wab = wat[:]
                pstp = wab.ap[0][0]
                for br in range(2):
                    for jx in range(3):
                        in0 = bass.AP(wab.tensor,
                                      wab.offset + jx * 36 + 18 * br,
                                      [[pstp, 128], [0, 3], [3, 3], [1, 3]])
                        in1 = bass.AP(wab.tensor, wab.offset + 9 + 18 * br,
                                      [[pstp, 128], [36, 3], [1, 3], [3, 3]])
                        wv = wpt[:, t, 81 * br + jx * 27:
                                 81 * br + (jx + 1) * 27]
                        outap = bass.AP(wv.tensor, wv.offset,
                                        [[wv.ap[0][0], 128], [9, 3], [3, 3],
                                         [1, 3]])
                        nc.vector.tensor_tensor(outap, in0, in1, Alu.mult)

        # -------- phase 4: restage + modulate + GEMMs + output ---------
        for kb in range(NBLK):
            wpt = wpt_all[kb]
            xoffs = [xoff_pool.tile([128, TPB, 10, 64], bf16, tag="xoff",
                                    name=f"xoff_{kb}_{i}")
                     for i in range(2)]
            for xoff in xoffs:
                nc.gpsimd.memset(xoff[:, :, 9, :], 0.0)
            for hb in range(2):
                x25 = x25_pool.tile([128, 4, 25, 64], bf16, tag="x25")
                base_ho = 4 * kb + 2 * hb
                for s in range(25):
                    dx, dy = s // 5 - 2, s % 5 - 2
                    for thi in range(2):
                        src_off = ((base_ho + thi + dx + 2) * QS
                                   + dy + 2) * C
                        src = bass.AP(xsT_d, src_off,
                                      [[C, 128], [128 * C, 2], [1, C]])
                        nc.sync.dma_start(x25[:, 2 * thi:2 * thi + 2, s, :],
                                          src)
                for t4 in range(4):
                    tblk = 4 * hb + t4
                    for br in range(2):
                        xoff = xoffs[br]
                        for n in range(9):
                            a, b = n // 3, n % 3
                            dst = xoff[:, tblk, n, :]
                            for jx in range(3):
                                for jy in range(3):
                                    s = (a + jx) * 5 + (b + jy)
                                    col = (81 * br + jx * 27 + jy * 9
                                           + a * 3 + b)
                                    wv = wpt[:, tblk, col:col + 1]
                                    src = x25[:, t4, s, :]
                                    if jx == 0 and jy == 0:
                                        # per-partition-scalar multiply on
                                        # the otherwise-idle ScalarEngine
                                        nc.scalar.activation(
                                            dst, src, Act.Identity,
                                            scale=wv)
                                    else:
                                        nc.vector.scalar_tensor_tensor(
                                            dst, src, wv, dst,
                                            Alu.mult, Alu.add)

            cat1 = cat_pool.tile([128, BLK], bf16, tag="cat1")
            cat2 = cat_pool.tile([64, BLK], bf16, tag="cat2")
            nc.sync.dma_start(cat1[0:64, :],
                              cm_ap[:, kb * BLK:(kb + 1) * BLK])
            for br in range(2):
                xoff = xoffs[br]
                for t in range(TPB):
                    ps = ps_g.tile([128, 128], f32, tag="psg")
                    for ck in range(5):
                        xt = xofft_pool.tile([128, 128], bf16, tag="xofft")
                        nc.sync.dma_start(xt[:], xoff[:, t, 2 * ck:2 * ck + 2, :],
                                          transpose=True)
                        if br == 0:
                            nc.tensor.matmul(ps[:], wcx_sb[:, ck, :], xt[:],
                                             start=(ck == 0), stop=(ck == 4))
                        else:
                            nc.tensor.matmul(ps[0:64, :], wcy_sb[:, ck, :],
                                             xt[:],
                                             start=(ck == 0), stop=(ck == 4))
                    if br == 0:
                        nc.scalar.activation(
                            cat1[64:128, t * 128:(t + 1) * 128],
                            ps[64:128, :], Act.Copy)
                    else:
                        nc.scalar.activation(
                            cat2[0:64, t * 128:(t + 1) * 128],
                            ps[0:64, :], Act.Copy)

            xc_t = io_pool.tile([64, BLK], f32, tag="xc")
            nc.sync.dma_start(xc_t[:], D["xc"].ap()[:, kb * BLK:(kb + 1) * BLK])
            out_t = io_pool.tile([64, BLK], f32, tag="out")
            for t in range(TPB):
                pso = ps_o.tile([64, 128], f32, tag="pso")
                nc.tensor.matmul(pso[:], wo1_sb[:],
                                 cat1[:, t * 128:(t + 1) * 128],
                                 start=True, stop=False)
                nc.tensor.matmul(pso[:], wo2_sb[:],
                                 cat2[:, t * 128:(t + 1) * 128],
                                 start=False, stop=True)
                nc.vector.scalar_tensor_tensor(
                    out_t[:, t * 128:(t + 1) * 128], pso[:], bo_sb[:],
                    xc_t[:, t * 128:(t + 1) * 128], Alu.add, Alu.add)
            nc.sync.dma_start(D["out"].ap()[:, kb * BLK:(kb + 1) * BLK],
                              out_t[:])


def _build_program():
    import concourse.bass as bass
    import concourse.mybir as mybir
    import concourse.tile as tile
    from concourse import bacc

    f32 = mybir.dt.float32
    bf16 = mybir.dt.bfloat16

    nc = bacc.Bacc("TRN2", target_bir_lowering=False, debug=False,
                   num_devices=N_CORES)

    D = {}
    D["xs"] = nc.dram_tensor("xs", [C, RS, QS], bf16, kind="ExternalInput")
    D["xsT"] = nc.dram_tensor("xsT", [P2, C], bf16, kind="ExternalInput")
    D["xc"] = nc.dram_tensor("xc", [C, POS], f32, kind="ExternalInput")
    D["wconv"] = nc.dram_tensor("wconv", [C, 9, 128], bf16,
                                kind="ExternalInput")
    D["bconv"] = nc.dram_tensor("bconv", [128, 1], f32, kind="ExternalInput")
    D["flag"] = nc.dram_tensor("flag", [128, 4], f32, kind="ExternalInput")
    D["wcx"] = nc.dram_tensor("wcx", [5, 128, 128], bf16,
                              kind="ExternalInput")
    D["wcy"] = nc.dram_tensor("wcy", [5, 128, 64], bf16,
                              kind="ExternalInput")
    D["wo1"] = nc.dram_tensor("wo1", [128, 64], bf16, kind="ExternalInput")
    D["wo2"] = nc.dram_tensor("wo2", [64, 64], bf16, kind="ExternalInput")
    D["bo"] = nc.dram_tensor("bo", [64, 1], f32, kind="ExternalInput")
    D["ident"] = nc.dram_tensor("ident", [128, 128], bf16,
                                kind="ExternalInput")
    D["out"] = nc.dram_tensor("out", [C, POS], f32, kind="ExternalOutput")
    D["cm_scratch"] = nc.dram_tensor("cm_scratch", [C, POS], bf16)

    with tile.TileContext(nc) as tc:
        _emit_device_program(tc, nc, bass, mybir, D)
    nc.compile()
    return nc


# ====================== host side ======================

_PNX = np.arange(9) // 3 - 1
_PNY = np.arange(9) % 3 - 1


def _bf(a):
    return np.asarray(a).astype(BF16).astype(np.float32)


def _host_offsets(xb, wp, bp):
    """Offsets for one image, simulating the device's bf16 conv."""
    xp = np.zeros((C, H + 2, W + 2), np.float32)
    xp[:, 1:1 + H, 1:1 + W] = xb
    xp = _bf(xp)
    off = np.zeros((2 * 9, H, W), np.float32)
    for kx in range(3):
        for ky in range(3):
            off += np.tensordot(_bf(wp[:, :, kx, ky]),
                                xp[:, kx:kx + H, ky:ky + W], axes=1)
    return _bf(off + bp[:, None, None])


def _branch_correction(xb, off, wc):
    """Exact sparse correction for |off|>1 taps: wc . (true - clamped)."""
    offx, offy = off[:9], off[9:]
    xp = np.zeros((C, H + 2, W + 2), np.float32)
    xp[:, 1:1 + H, 1:1 + W] = xb
    Hp = H + 2
    wcr = wc.reshape(wc.shape[0], C, 9)
    delta = {}
    bad = np.argwhere((np.abs(offx) > 1) | (np.abs(offy) > 1))
    for n, h, w in bad:
        def sample(ox, oy):
            px = h + 1 + _PNX[n] + ox
            py = w + 1 + _PNY[n] + oy
            fx, fy = np.floor(px), np.floor(py)
            qltx = np.clip(fx, 0, Hp - 1); qlty = np.clip(fy, 0, Hp - 1)
            qrbx = np.clip(fx + 1, 0, Hp - 1)
            qrby = np.clip(fy + 1, 0, Hp - 1)
            pxc = np.clip(px, 0, Hp - 1); pyc = np.clip(py, 0, Hp - 1)
            glt = (1 + qltx - pxc) * (1 + qlty - pyc)
            grb = (1 - qrbx + pxc) * (1 - qrby + pyc)
            glb = (1 + qltx - pxc) * (1 - qrby + pyc)
            grt = (1 - qrbx + pxc) * (1 + qlty - pyc)
            return (glt * xp[:, int(qltx), int(qlty)]
                    + grb * xp[:, int(qrbx), int(qrby)]
                    + glb * xp[:, int(qltx), int(qrby)]
                    + grt * xp[:, int(qrbx), int(qlty)])

        ox, oy = offx[n, h, w], offy[n, h, w]
        dv = sample(ox, oy) - sample(np.clip(ox, -1, 1), np.clip(oy, -1, 1))
        key = (h, w)
        delta.setdefault(key, np.zeros(wc.shape[0], np.float32))
        delta[key] += wcr[:, :, n] @ dv
    return delta


def _apply_host_corrections(inputs, out):
    wo = inputs["wo"][:, :, 0, 0]
    for b in range(B):
        xb = inputs["x"][b]
        for br, blk in (("x", slice(0, 64)), ("y", slice(128, 192))):
            off = _host_offsets(xb, inputs[f"wp_{br}"], inputs[f"bp_{br}"])
            delta = _branch_correction(xb, off, inputs[f"wc_{br}"])
            for (h, w), dc in delta.items():
                out[b, :, h, w] += wo[:, blk] @ dc


def _prep_core_inputs(inputs, core):
    x = inputs["x"]
    b, slab = core // 4, core % 4
    h0 = slab * HS

    xp2 = np.zeros((C, H + 4, W + 4), np.float32)
    xp2[:, 2:2 + H, 2:2 + W] = x[b]
    xs = xp2[:, h0:h0 + RS, :]                     # [C, 68, 260]
    xs_bf = xs.astype(BF16)
    xsT = np.ascontiguousarray(xs_bf.reshape(C, P2).T)   # [P2, C]
    xc = np.ascontiguousarray(
        x[b, :, h0:h0 + HS, :].reshape(C, POS)).astype(np.float32)

    perm_y = np.array([0, 3, 6, 1, 4, 7, 2, 5, 8])
    wconv = np.zeros((C, 9, 128), np.float32)
    for k in range(9):
        kx, ky = k // 3, k % 3
        wconv[:, k, 0:64] = inputs["wm"][:, :, kx, ky].T
        wconv[:, k, 64:73] = inputs["wp_x"][0:9, :, kx, ky].T
        wconv[:, k, 73:82] = inputs["wp_x"][9 + perm_y, :, kx, ky].T
        wconv[:, k, 82:91] = inputs["wp_y"][0:9, :, kx, ky].T
        wconv[:, k, 91:100] = inputs["wp_y"][9 + perm_y, :, kx, ky].T
    bconv = np.zeros((128, 1), np.float32)
    bconv[0:64, 0] = inputs["bm"]
    bconv[64:73, 0] = inputs["bp_x"][0:9]
    bconv[73:82, 0] = inputs["bp_x"][9 + perm_y]
    bconv[82:91, 0] = inputs["bp_y"][0:9]
    bconv[91:100, 0] = inputs["bp_y"][9 + perm_y]

    flag_lo = 1.0 if slab == 0 else 0.0
    flag_hi = 1.0 if slab == 3 else 0.0
    flag = np.zeros((128, 4), np.float32)
    for base in (64, 82):                 # offx rows (tap-natural order)
        flag[base + 0:base + 3, 0] = flag_lo    # pnx=-1 -> x-lo
        flag[base + 6:base + 9, 1] = flag_hi    # pnx=+1 -> x-hi
    for base in (73, 91):                 # offy rows (pny-major order)
        flag[base + 0:base + 3, 2] = 1.0        # pny=-1 -> y-lo
        flag[base + 6:base + 9, 3] = 1.0        # pny=+1 -> y-hi

    def wc_chunks(wc, col0, width):
        wcr = wc.reshape(64, 64, 9)              # [o, c, n]
        m = np.zeros((640, width), np.float32)
        for n in range(9):
            m[n * 64:(n + 1) * 64, col0:col0 + 64] = wcr[:, :, n].T
        return m.reshape(5, 128, width)

    wcx = wc_chunks(inputs["wc_x"], 64, 128)
    wcy = wc_chunks(inputs["wc_y"], 0, 64)

    wo = inputs["wo"][:, :, 0, 0]                # [64, 192] inputs cx,cm,cy
    wo1 = np.zeros((128, 64), np.float32)
    wo1[0:64] = wo[:, 64:128].T                  # cm rows
    wo1[64:128] = wo[:, 0:64].T                  # cx rows
    wo2 = np.ascontiguousarray(wo[:, 128:192].T)
    bo = inputs["bo"].reshape(64, 1).astype(np.float32)
    ident = np.zeros((128, 128), np.float32)
    ident[64:100, 0:36] = np.eye(36)
    ident = ident.astype(BF16)

    return {
        "xs": xs_bf, "xsT": xsT, "xc": xc,
        "wconv": wconv.astype(BF16), "bconv": bconv, "flag": flag,
        "wcx": wcx.astype(BF16), "wcy": wcy.astype(BF16),
        "wo1": wo1.astype(BF16), "wo2": wo2.astype(BF16), "bo": bo,
        "ident": ident,
    }


def kernel(**inputs):
    from concourse.bass_utils import run_bass_kernel_spmd

    inputs = {k: np.asarray(v) for k, v in inputs.items()}
    if "nc" not in _CACHE:
        _CACHE["nc"] = _build_program()
    nc = _CACHE["nc"]

    in_maps = [_prep_core_inputs(inputs, c) for c in range(N_CORES)]
    res = run_bass_kernel_spmd(nc, in_maps, core_ids=list(range(N_CORES)))
    _CACHE["last_result"] = res

    out = np.empty((B, C, H, W), np.float32)
    for core in range(N_CORES):
        b, slab = core // 4, core % 4
        o = res.results[core]["out"].reshape(C, HS, W)
        out[b, :, slab * HS:(slab + 1) * HS, :] = o
    _apply_host_corrections(inputs, out)
    return out

